# revision 6
# baseline (speedup 1.0000x reference)
"""Distributed spherical self-attention (DistributedAttentionS2) on 8 TRN2
NeuronCores.

Sharding: head-parallel (tensor parallel). 8 heads, 8 cores, one head per
core, no collectives. Each core receives the full (replicated) input grid
plus its head's slices of the QKV/proj weights, computes

    U_h = p_w[:, h] @ (sum_m exp(s_nm) * v_m)    (un-normalized)
    r_h = sum_m exp(s_nm)                        (softmax denominators)

and the host combines:  out = sum_h U_h / r_h  (+ bias terms).

Per-core kernel structure (N = 46*90 = 4140 pixels, dk = 32):
  - Q/K projections write "replicated" layouts: Qrep/Krep [128, N] with the
    head's 32 channels duplicated at partition bases 0 and 64, plus an
    augmentation row (Q row 32/96 = 1, K row 32/96 = centered log-quadrature
    weights * sqrt(dk)) so the additive softmax bias rides inside the score
    matmul's 33-deep contraction.
  - Scores S^T [keys, queries] via f32r matmuls, 2-way row-tiled (bases
    0/64), 33 key-chunks x 9 query-chunks of 460.
  - exp on ScalarE directly from PSUM (scale = 1/sqrt(dk) folded in),
    bf16 output. The global bias shift cancels in U/r.
  - attnV: V^T with a ones column (rowsums for free), 2-way col-tiled
    (two query-chunks at PSUM partition bases 0/64).
  - p-projection in-kernel; normalization is a host-side division.
"""

import math

import numpy as np

HEADS = 8
C = 256
DK = 32
HLAT, WLON = 46, 90
N = HLAT * WLON  # 4140
NKC = 33  # key chunks of 128
NPAD = NKC * 128  # 4224
QCH = 460
NQC = 9  # 9 * 460 == 4140
SCALE = 1.0 / math.sqrt(DK)
NEG = -1.0e30

_cache = {}


def _build_nc():
    from contextlib import ExitStack

    import concourse.mybir as mybir
    import concourse.tile as tile
    from concourse import bacc

    f32 = mybir.dt.float32
    f32r = mybir.dt.float32r
    bf16 = mybir.dt.bfloat16

    nc = bacc.Bacc("TRN2", target_bir_lowering=False, debug=False)

    xd = nc.dram_tensor("x", [2, 128, NPAD], f32r, kind="ExternalInput")
    wqt = nc.dram_tensor("wqt", [2, 128, 128], f32r, kind="ExternalInput")
    wkt = nc.dram_tensor("wkt", [2, 128, 128], f32r, kind="ExternalInput")
    wvt = nc.dram_tensor("wvt", [2, 128, 32], f32r, kind="ExternalInput")
    pwt = nc.dram_tensor("pwt", [128, 256], f32r, kind="ExternalInput")
    lqw = nc.dram_tensor("lqw", [1, NPAD], f32r, kind="ExternalInput")
    qone = nc.dram_tensor("qone", [1, N], f32r, kind="ExternalInput")
    ud = nc.dram_tensor("u", [2, 128, N], f32, kind="ExternalOutput")
    rd = nc.dram_tensor("r", [1, N], f32r, kind="ExternalOutput")

    with tile.TileContext(nc) as tc, ExitStack() as ctx:
        sing = ctx.enter_context(tc.tile_pool(name="sing", bufs=1))
        ets = ctx.enter_context(tc.tile_pool(name="ets", bufs=3))
        ous = ctx.enter_context(tc.tile_pool(name="ous", bufs=2))
        us = ctx.enter_context(tc.tile_pool(name="us", bufs=2))
        ps_s = ctx.enter_context(tc.tile_pool(name="ps_s", bufs=2, space="PSUM"))
        ps_o = ctx.enter_context(tc.tile_pool(name="ps_o", bufs=1, space="PSUM"))
        ps_u = ctx.enter_context(tc.tile_pool(name="ps_u", bufs=1, space="PSUM"))

        sb_x = sing.tile([128, 2, NPAD], f32r)
        sb_wqt = sing.tile([128, 2, 128], f32r)
        sb_wkt = sing.tile([128, 2, 128], f32r)
        sb_wvt = sing.tile([128, 2, 32], f32r)
        sb_pwt = sing.tile([128, 256], f32r)
        sb_q = sing.tile([128, N], f32r)
        sb_k = sing.tile([128, NPAD], f32r)
        sb_vt = sing.tile([128, NKC, 33], bf16)

        for cc in range(2):
            nc.sync.dma_start(out=sb_x[:, cc, :], in_=xd[cc])
            nc.sync.dma_start(out=sb_wqt[:, cc, :], in_=wqt[cc])
            nc.sync.dma_start(out=sb_wkt[:, cc, :], in_=wkt[cc])
            nc.sync.dma_start(out=sb_wvt[:, cc, :], in_=wvt[cc])
        nc.sync.dma_start(out=sb_pwt[:], in_=pwt[:])

        # ---- phase A: projections ----
        # Q projections: chunks of 460 columns. K projections additionally
        # cover the zero-padded tail so padded-key columns land as 0.
        for qc in range(NQC):
            sl = slice(qc * QCH, (qc + 1) * QCH)
            pq = ps_s.tile([128, QCH], f32, tag="s")
            for cc in range(2):
                nc.tensor.matmul(
                    pq[:, :],
                    sb_wqt[:, cc, :],
                    sb_x[:, cc, sl],
                    start=(cc == 0),
                    stop=(cc == 1),
                )
            nc.vector.tensor_copy(out=sb_q[:, sl], in_=pq[:, :])
        k_slices = [slice(qc * QCH, (qc + 1) * QCH) for qc in range(NQC)]
        k_slices.append(slice(N, NPAD))
        for sl in k_slices:
            w = sl.stop - sl.start
            pk = ps_s.tile([128, QCH], f32, tag="s")
            for cc in range(2):
                nc.tensor.matmul(
                    pk[:, 0:w],
                    sb_wkt[:, cc, :],
                    sb_x[:, cc, sl],
                    start=(cc == 0),
                    stop=(cc == 1),
                )
            nc.vector.tensor_copy(out=sb_k[:, sl], in_=pk[:, 0:w])

        # Augmentation rows: Q rows 32/96 = 1.0, K rows 32/96 = lqw (these
        # writes overlap the projection copies, so Tile orders them after).
        nc.sync.dma_start(out=sb_q[32:33, :], in_=qone[:])
        nc.sync.dma_start(out=sb_q[96:97, :], in_=qone[:])
        nc.sync.dma_start(out=sb_k[32:33, :], in_=lqw[:])
        nc.sync.dma_start(out=sb_k[96:97, :], in_=lqw[:])

        # V^T (pixels on partitions) + ones column.
        pv = ps_s.tile([128, NKC * 32], f32, tag="s")
        for kc in range(NKC):
            for cc in range(2):
                nc.tensor.matmul(
                    pv[:, kc * 32 : (kc + 1) * 32],
                    sb_x[:, cc, kc * 128 : (kc + 1) * 128],
                    sb_wvt[:, cc, :],
                    start=(cc == 0),
                    stop=(cc == 1),
                )
        nc.vector.tensor_copy(
            out=sb_vt[:, :, 0:32],
            in_=pv[:, :].rearrange("p (a b) -> p a b", b=32),
        )
        nc.gpsimd.memset(sb_vt[:, :, 32:33], 1.0)

        # ---- phases B/C per query chunk ----
        et_tiles = []

        def scores_and_exp(qc):
            et = ets.tile([128, NKC, QCH], bf16, tag="et")
            qsl = slice(qc * QCH, (qc + 1) * QCH)
            for g in range(11):
                pg = ps_s.tile([128, 3, 512], f32, tag="s")
                for t in range(3):
                    kc = 3 * g + t
                    base = 64 * (kc % 2)
                    nc.tensor.matmul(
                        pg[:, t, 0:QCH],
                        sb_k[base : base + 33, kc * 128 : (kc + 1) * 128],
                        sb_q[base : base + 33, qsl],
                    )
                nc.scalar.activation(
                    out=et[:, 3 * g : 3 * g + 3, :],
                    in_=pg[:, :, 0:QCH],
                    func=mybir.ActivationFunctionType.Exp,
                    scale=SCALE,
                    bias=0.0,
                )
            return et

        def attnv_pair(jlo, pair):
            # pair==2: col-tiled strips at partition bases 0 (qchunk jlo) and
            # 64 (qchunk jlo+1) accumulating in one PSUM bank.
            po = ps_o.tile([128, 512], f32, tag="o")
            for kc in range(NKC):
                for s in range(pair):
                    base = 64 * s
                    nc.tensor.matmul(
                        po[base : base + 33, 0:QCH],
                        sb_vt[:, kc, :],
                        et_tiles[jlo + s][:, kc, :],
                        start=(kc == 0),
                        stop=(kc == NKC - 1),
                        skip_group_check=True,
                    )
            ou = ous.tile([128, QCH], f32r, tag="ou")
            for s in range(pair):
                base = 64 * s
                qc = jlo + s
                nc.vector.tensor_copy(
                    out=ou[base : base + 33, :], in_=po[base : base + 33, 0:QCH]
                )
                nc.sync.dma_start(
                    out=rd[0:1, qc * QCH : (qc + 1) * QCH],
                    in_=ou[base + 32 : base + 33, :],
                )
            # p-projection for each strip
            for s in range(pair):
                base = 64 * s
                qc = jlo + s
                for mc in range(2):
                    pu = ps_u.tile([128, 512], f32, tag="u")
                    nc.tensor.matmul(
                        pu[:, 0:QCH],
                        sb_pwt[base : base + 32, mc * 128 : (mc + 1) * 128],
                        ou[base : base + 32, :],
                    )
                    ut = us.tile([128, QCH], f32, tag="u")
                    nc.vector.tensor_copy(out=ut[:], in_=pu[:, 0:QCH])
                    nc.sync.dma_start(
                        out=ud[mc, :, qc * QCH : (qc + 1) * QCH], in_=ut[:]
                    )

        for qc in range(NQC):
            et_tiles.append(scores_and_exp(qc))
            if qc % 2 == 1:
                attnv_pair(qc - 1, 2)
        attnv_pair(NQC - 1, 1)

    nc.compile()
    return nc


def _host_inputs(query, q_w, k_w, v_w, p_w, q_b, k_b, log_qw):
    xf = np.ascontiguousarray(
        np.asarray(query, dtype=np.float32).reshape(C, N)
    )
    x_pad = np.zeros((2, 128, NPAD), np.float32)
    x_pad[0, :, :N] = xf[0:128]
    x_pad[1, :, :N] = xf[128:256]

    lq = np.asarray(log_qw, dtype=np.float32).reshape(N)
    lq_c = (lq - lq.max()) * math.sqrt(DK)

    in_maps = []
    for h in range(HEADS):
        hs = slice(DK * h, DK * (h + 1))
        wq_h = np.asarray(q_w, np.float32)[hs]  # [32, 256]
        wk_h = np.asarray(k_w, np.float32)[hs]
        wv_h = np.asarray(v_w, np.float32)[hs]
        pw_h = np.asarray(p_w, np.float32)[:, hs]  # [256, 32]

        wst_q = np.zeros((128, C), np.float32)
        wst_q[0:32] = wq_h
        wst_q[64:96] = wq_h
        wst_k = np.zeros((128, C), np.float32)
        wst_k[0:32] = wk_h
        wst_k[64:96] = wk_h
        wqt = np.ascontiguousarray(
            wst_q.T.reshape(2, 128, 128).transpose(0, 1, 2)
        )
        # wst.T is [256, 128]; chunk rows into two [128, 128] blocks
        wqt = np.ascontiguousarray(wst_q.T.reshape(2, 128, 128))
        wkt = np.ascontiguousarray(wst_k.T.reshape(2, 128, 128))
        wvt = np.ascontiguousarray(wv_h.T.reshape(2, 128, 32))

        pwt = np.zeros((128, 256), np.float32)
        pwt[0:32] = pw_h.T
        pwt[64:96] = pw_h.T

        aug = lq_c.copy()
        qb_h = np.asarray(q_b, np.float32)[hs]
        if np.any(qb_h):
            Kh = wk_h @ xf + np.asarray(k_b, np.float32)[hs][:, None]
            aug = aug + qb_h @ Kh
        lqw_row = np.full((1, NPAD), NEG, np.float32)
        lqw_row[0, :N] = aug

        in_maps.append(
            {
                "x": x_pad,
                "wqt": wqt,
                "wkt": wkt,
                "wvt": wvt,
                "pwt": pwt,
                "lqw": lqw_row,
                "qone": np.ones((1, N), np.float32),
            }
        )
    return in_maps


def kernel(query, q_w, q_b, k_w, k_b, v_w, v_b, p_w, p_b, log_qw, _res=None):
    from concourse.bass_utils import run_bass_kernel_spmd

    if "nc" not in _cache:
        _cache["nc"] = _build_nc()
    nc = _cache["nc"]

    in_maps = _host_inputs(query, q_w, k_w, v_w, p_w, q_b, k_b, log_qw)
    res = run_bass_kernel_spmd(nc, in_maps, core_ids=list(range(8)))
    if _res is not None:
        _res.append(res)

    acc = np.zeros((C, N), np.float64)
    for h in range(HEADS):
        u = res.results[h]["u"].astype(np.float64).reshape(C, N)
        r = res.results[h]["r"].astype(np.float64).reshape(N)
        acc += u / r[None, :]

    acc += (np.asarray(p_w, np.float64) @ np.asarray(v_b, np.float64))[:, None]
    acc += np.asarray(p_b, np.float64)[:, None]
    return acc.astype(np.float32).reshape(1, C, HLAT, WLON)


# revision 7
# speedup vs baseline: 1.1269x; 1.1269x over previous
"""Distributed spherical self-attention (DistributedAttentionS2) on 8 TRN2
NeuronCores.

Sharding: head-parallel (tensor parallel). 8 heads, 8 cores, one head per
core, no collectives. Each core receives the full (replicated) input grid
plus its head's slices of the QKV/proj weights, computes

    U_h = p_w[:, h] @ (sum_m qw_m exp(s_nm) * v_m)    (un-normalized)
    r_h = sum_m qw_m exp(s_nm)                        (softmax denominators)

and the host combines:  out = sum_h U_h / r_h  (+ bias terms).

The additive log-quadrature bias on the scores is algebraically a
per-key multiplicative weight qw_m on exp(s); it is folded into V (and
into the rowsum column) as a diagonal scale, which keeps the score
matmul contraction at 32 and enables 4-way PE row tiling.

Per-core kernel structure (N = 46*90 = 4140 pixels, dk = 32):
  - Q/K projections write 4-stacked layouts Qrep/Krep [128, N]: the head's
    32 channels replicated at partition bases 0/32/64/96.
  - Scores S^T [keys, queries] via f32r matmuls, 4-way row-tiled,
    33 key-chunks x 9 query-chunks of 460.
  - exp on ScalarE directly from PSUM (scale = 1/sqrt(dk) folded in),
    bf16 output.
  - attnV: V^T scaled by qw with a qw column appended (weighted rowsums
    for free), 2-way col-tiled (two query-chunks at PSUM bases 0/64).
  - p-projection in-kernel; normalization is a host-side division.
"""

import math

import numpy as np

HEADS = 8
C = 256
DK = 32
HLAT, WLON = 46, 90
N = HLAT * WLON  # 4140
NKC = 33  # key chunks of 128
NPAD = NKC * 128  # 4224
QCH = 460
NQC = 9  # 9 * 460 == 4140
SCALE = 1.0 / math.sqrt(DK)

_cache = {}


def _build_nc():
    from contextlib import ExitStack

    import concourse.mybir as mybir
    import concourse.tile as tile
    from concourse import bacc

    f32 = mybir.dt.float32
    f32r = mybir.dt.float32r
    bf16 = mybir.dt.bfloat16

    nc = bacc.Bacc("TRN2", target_bir_lowering=False, debug=False)

    xd = nc.dram_tensor("x", [2, 128, NPAD], f32r, kind="ExternalInput")
    wqt = nc.dram_tensor("wqt", [2, 128, 128], f32r, kind="ExternalInput")
    wkt = nc.dram_tensor("wkt", [2, 128, 128], f32r, kind="ExternalInput")
    wvt = nc.dram_tensor("wvt", [2, 128, 32], f32r, kind="ExternalInput")
    pwt = nc.dram_tensor("pwt", [128, 256], f32r, kind="ExternalInput")
    qwd = nc.dram_tensor("qwd", [128, NKC], f32, kind="ExternalInput")
    ud = nc.dram_tensor("u", [2, 128, N], f32, kind="ExternalOutput")
    rd = nc.dram_tensor("r", [1, N], f32r, kind="ExternalOutput")

    with tile.TileContext(nc) as tc, ExitStack() as ctx:
        sing = ctx.enter_context(tc.tile_pool(name="sing", bufs=1))
        ets = ctx.enter_context(tc.tile_pool(name="ets", bufs=3))
        ous = ctx.enter_context(tc.tile_pool(name="ous", bufs=2))
        us = ctx.enter_context(tc.tile_pool(name="us", bufs=2))
        ps_s = ctx.enter_context(tc.tile_pool(name="ps_s", bufs=2, space="PSUM"))
        ps_o = ctx.enter_context(tc.tile_pool(name="ps_o", bufs=1, space="PSUM"))
        ps_u = ctx.enter_context(tc.tile_pool(name="ps_u", bufs=1, space="PSUM"))

        sb_x = sing.tile([128, 2, NPAD], f32r)
        sb_wqt = sing.tile([128, 2, 128], f32r)
        sb_wkt = sing.tile([128, 2, 128], f32r)
        sb_wvt = sing.tile([128, 2, 32], f32r)
        sb_pwt = sing.tile([128, 256], f32r)
        sb_qw = sing.tile([128, NKC], f32)
        sb_q = sing.tile([128, N], f32r)
        sb_k = sing.tile([128, NPAD], f32r)
        sb_vt = sing.tile([128, NKC, 33], bf16)

        for cc in range(2):
            nc.sync.dma_start(out=sb_x[:, cc, :], in_=xd[cc])
            nc.sync.dma_start(out=sb_wqt[:, cc, :], in_=wqt[cc])
            nc.sync.dma_start(out=sb_wkt[:, cc, :], in_=wkt[cc])
            nc.sync.dma_start(out=sb_wvt[:, cc, :], in_=wvt[cc])
        nc.sync.dma_start(out=sb_pwt[:], in_=pwt[:])
        nc.sync.dma_start(out=sb_qw[:], in_=qwd[:])

        # ---- phase A: projections ----
        # Q projections: chunks of 460 columns. K projections additionally
        # cover the zero-padded tail so padded-key columns land as 0.
        for qc in range(NQC):
            sl = slice(qc * QCH, (qc + 1) * QCH)
            pq = ps_s.tile([128, QCH], f32, tag="s")
            for cc in range(2):
                nc.tensor.matmul(
                    pq[:, :],
                    sb_wqt[:, cc, :],
                    sb_x[:, cc, sl],
                    start=(cc == 0),
                    stop=(cc == 1),
                )
            nc.vector.tensor_copy(out=sb_q[:, sl], in_=pq[:, :])
        k_slices = [slice(qc * QCH, (qc + 1) * QCH) for qc in range(NQC)]
        k_slices.append(slice(N, NPAD))
        for sl in k_slices:
            w = sl.stop - sl.start
            pk = ps_s.tile([128, QCH], f32, tag="s")
            for cc in range(2):
                nc.tensor.matmul(
                    pk[:, 0:w],
                    sb_wkt[:, cc, :],
                    sb_x[:, cc, sl],
                    start=(cc == 0),
                    stop=(cc == 1),
                )
            nc.vector.tensor_copy(out=sb_k[:, sl], in_=pk[:, 0:w])

        # V^T (pixels on partitions): copy, append ones, scale by qw so the
        # last column holds qw (weighted-rowsum denominators).
        pv = ps_s.tile([128, NKC * 32], f32, tag="s")
        for kc in range(NKC):
            for cc in range(2):
                nc.tensor.matmul(
                    pv[:, kc * 32 : (kc + 1) * 32],
                    sb_x[:, cc, kc * 128 : (kc + 1) * 128],
                    sb_wvt[:, cc, :],
                    start=(cc == 0),
                    stop=(cc == 1),
                )
        nc.vector.tensor_copy(
            out=sb_vt[:, :, 0:32],
            in_=pv[:, :].rearrange("p (a b) -> p a b", b=32),
        )
        nc.gpsimd.memset(sb_vt[:, :, 32:33], 1.0)
        for kc in range(NKC):
            nc.vector.tensor_scalar_mul(
                out=sb_vt[:, kc, :],
                in0=sb_vt[:, kc, :],
                scalar1=sb_qw[:, kc : kc + 1],
            )

        # ---- phases B/C per query chunk ----
        et_tiles = []

        def scores_and_exp(qc):
            et = ets.tile([128, NKC, QCH], bf16, tag="et")
            qsl = slice(qc * QCH, (qc + 1) * QCH)
            for g in range(11):
                pg = ps_s.tile([128, 3, 512], f32, tag="s")
                for t in range(3):
                    kc = 3 * g + t
                    base = 32 * (kc % 4)
                    nc.tensor.matmul(
                        pg[:, t, 0:QCH],
                        sb_k[base : base + 32, kc * 128 : (kc + 1) * 128],
                        sb_q[base : base + 32, qsl],
                        tile_position=(base, 0),
                    )
                nc.scalar.activation(
                    out=et[:, 3 * g : 3 * g + 3, :],
                    in_=pg[:, :, 0:QCH],
                    func=mybir.ActivationFunctionType.Exp,
                    scale=SCALE,
                    bias=0.0,
                )
            return et

        def attnv_pair(jlo, pair):
            # col-tiled strips at partition bases 0 (qchunk jlo) and 64
            # (qchunk jlo+1) accumulating in one PSUM bank.
            po = ps_o.tile([128, 512], f32, tag="o")
            for kc in range(NKC):
                for s in range(pair):
                    base = 64 * s
                    nc.tensor.matmul(
                        po[base : base + 33, 0:QCH],
                        sb_vt[:, kc, :],
                        et_tiles[jlo + s][:, kc, :],
                        start=(kc == 0),
                        stop=(kc == NKC - 1),
                        skip_group_check=True,
                    )
            ou = ous.tile([128, QCH], f32r, tag="ou")
            for s in range(pair):
                base = 64 * s
                qc = jlo + s
                nc.vector.tensor_copy(
                    out=ou[base : base + 33, :], in_=po[base : base + 33, 0:QCH]
                )
                nc.sync.dma_start(
                    out=rd[0:1, qc * QCH : (qc + 1) * QCH],
                    in_=ou[base + 32 : base + 33, :],
                )
            # p-projection for each strip
            for s in range(pair):
                base = 64 * s
                qc = jlo + s
                for mc in range(2):
                    pu = ps_u.tile([128, 512], f32, tag="u")
                    nc.tensor.matmul(
                        pu[:, 0:QCH],
                        sb_pwt[base : base + 32, mc * 128 : (mc + 1) * 128],
                        ou[base : base + 32, :],
                    )
                    ut = us.tile([128, QCH], f32, tag="u")
                    nc.vector.tensor_copy(out=ut[:], in_=pu[:, 0:QCH])
                    nc.sync.dma_start(
                        out=ud[mc, :, qc * QCH : (qc + 1) * QCH], in_=ut[:]
                    )

        for qc in range(NQC):
            et_tiles.append(scores_and_exp(qc))
            if qc % 2 == 1:
                attnv_pair(qc - 1, 2)
        attnv_pair(NQC - 1, 1)

    nc.compile()
    return nc


def _host_inputs(query, q_w, k_w, v_w, p_w, q_b, k_b, log_qw):
    xf = np.ascontiguousarray(
        np.asarray(query, dtype=np.float32).reshape(C, N)
    )
    x_pad = np.zeros((2, 128, NPAD), np.float32)
    x_pad[0, :, :N] = xf[0:128]
    x_pad[1, :, :N] = xf[128:256]

    lq = np.asarray(log_qw, dtype=np.float32).reshape(N).astype(np.float64)
    lq = lq - lq.max()  # global shift cancels in U/r

    in_maps = []
    for h in range(HEADS):
        hs = slice(DK * h, DK * (h + 1))
        wq_h = np.asarray(q_w, np.float32)[hs]  # [32, 256]
        wk_h = np.asarray(k_w, np.float32)[hs]
        wv_h = np.asarray(v_w, np.float32)[hs]
        pw_h = np.asarray(p_w, np.float32)[:, hs]  # [256, 32]

        wqt = np.ascontiguousarray(np.tile(wq_h, (4, 1)).T.reshape(2, 128, 128))
        wkt = np.ascontiguousarray(np.tile(wk_h, (4, 1)).T.reshape(2, 128, 128))
        wvt = np.ascontiguousarray(wv_h.T.reshape(2, 128, 32))

        pwt = np.zeros((128, 256), np.float32)
        pwt[0:32] = pw_h.T
        pwt[64:96] = pw_h.T

        lq_h = lq
        qb_h = np.asarray(q_b, np.float64)[hs]
        if np.any(qb_h):
            Kh = (
                np.asarray(k_w, np.float64)[hs] @ xf.astype(np.float64)
                + np.asarray(k_b, np.float64)[hs][:, None]
            )
            lq_h = lq + SCALE * (qb_h @ Kh)
        qw_pad = np.zeros(NPAD, np.float64)
        qw_pad[:N] = np.exp(lq_h)
        qwd = np.ascontiguousarray(
            qw_pad.reshape(NKC, 128).T.astype(np.float32)
        )

        in_maps.append(
            {
                "x": x_pad,
                "wqt": wqt,
                "wkt": wkt,
                "wvt": wvt,
                "pwt": pwt,
                "qwd": qwd,
            }
        )
    return in_maps


def kernel(query, q_w, q_b, k_w, k_b, v_w, v_b, p_w, p_b, log_qw, _res=None):
    from concourse.bass_utils import run_bass_kernel_spmd

    if "nc" not in _cache:
        _cache["nc"] = _build_nc()
    nc = _cache["nc"]

    in_maps = _host_inputs(query, q_w, k_w, v_w, p_w, q_b, k_b, log_qw)
    res = run_bass_kernel_spmd(nc, in_maps, core_ids=list(range(8)))
    if _res is not None:
        _res.append(res)

    acc = np.zeros((C, N), np.float64)
    for h in range(HEADS):
        u = res.results[h]["u"].astype(np.float64).reshape(C, N)
        r = res.results[h]["r"].astype(np.float64).reshape(N)
        acc += u / r[None, :]

    acc += (np.asarray(p_w, np.float64) @ np.asarray(v_b, np.float64))[:, None]
    acc += np.asarray(p_b, np.float64)[:, None]
    return acc.astype(np.float32).reshape(1, C, HLAT, WLON)


# revision 9
# speedup vs baseline: 1.1504x; 1.0208x over previous
"""Distributed spherical self-attention (DistributedAttentionS2) on 8 TRN2
NeuronCores.

Sharding: head-parallel (tensor parallel). 8 heads, 8 cores, one head per
core, no collectives. Each core receives the full (replicated) input grid
plus its head's slices of the QKV/proj weights, computes

    U_h = p_w[:, h] @ (sum_m qw_m exp(s_nm) * v_m)    (un-normalized)
    r_h = sum_m qw_m exp(s_nm)                        (softmax denominators)

and the host combines:  out = sum_h U_h / r_h  (+ bias terms).

The additive log-quadrature bias on the scores is algebraically a
per-key multiplicative weight qw_m on exp(s); it is folded into V (and
into the rowsum column) as a diagonal scale, which keeps the score
matmul contraction at 32 and enables 4-way PE row tiling.

Per-core kernel structure (N = 46*90 = 4140 pixels, dk = 32):
  - Q/K projections write 4-stacked layouts Qrep/Krep [128, N]: the head's
    32 channels replicated at partition bases 0/32/64/96.
  - Scores S^T [keys, queries] via f32r matmuls, 4-way row-tiled,
    33 key-chunks x 9 query-chunks of 460.
  - exp on ScalarE directly from PSUM (scale = 1/sqrt(dk) folded in),
    bf16 output.
  - attnV: V^T scaled by qw with a qw column appended (weighted rowsums
    for free), 2-way col-tiled (two query-chunks at PSUM bases 0/64).
  - p-projection in-kernel; normalization is a host-side division.
"""

import math

import numpy as np

HEADS = 8
C = 256
DK = 32
HLAT, WLON = 46, 90
N = HLAT * WLON  # 4140
NKC = 33  # key chunks of 128
NPAD = NKC * 128  # 4224
QCH = 460
NQC = 9  # 9 * 460 == 4140
SCALE = 1.0 / math.sqrt(DK)

_cache = {}


def _build_nc():
    from contextlib import ExitStack

    import concourse.mybir as mybir
    import concourse.tile as tile
    from concourse import bacc

    f32 = mybir.dt.float32
    f32r = mybir.dt.float32r
    bf16 = mybir.dt.bfloat16

    nc = bacc.Bacc("TRN2", target_bir_lowering=False, debug=False)

    xd = nc.dram_tensor("x", [2, 128, NPAD], f32r, kind="ExternalInput")
    wqt = nc.dram_tensor("wqt", [2, 128, 128], f32r, kind="ExternalInput")
    wkt = nc.dram_tensor("wkt", [2, 128, 128], f32r, kind="ExternalInput")
    wvt = nc.dram_tensor("wvt", [2, 128, 32], f32r, kind="ExternalInput")
    pwt = nc.dram_tensor("pwt", [128, 256], f32r, kind="ExternalInput")
    qwd = nc.dram_tensor("qwd", [128, NKC], f32, kind="ExternalInput")
    ud = nc.dram_tensor("u", [2, 128, N], f32, kind="ExternalOutput")
    rd = nc.dram_tensor("r", [1, N], f32r, kind="ExternalOutput")

    with tile.TileContext(nc) as tc, ExitStack() as ctx:
        sing = ctx.enter_context(tc.tile_pool(name="sing", bufs=1))
        ets = ctx.enter_context(tc.tile_pool(name="ets", bufs=3))
        ous = ctx.enter_context(tc.tile_pool(name="ous", bufs=2))
        us = ctx.enter_context(tc.tile_pool(name="us", bufs=2))
        ps_s = ctx.enter_context(tc.tile_pool(name="ps_s", bufs=2, space="PSUM"))
        ps_o = ctx.enter_context(tc.tile_pool(name="ps_o", bufs=1, space="PSUM"))
        ps_u = ctx.enter_context(tc.tile_pool(name="ps_u", bufs=1, space="PSUM"))

        sb_x = sing.tile([128, 2, NPAD], f32r)
        sb_wqt = sing.tile([128, 2, 128], f32r)
        sb_wkt = sing.tile([128, 2, 128], f32r)
        sb_wvt = sing.tile([128, 2, 32], f32r)
        sb_pwt = sing.tile([128, 256], f32r)
        sb_qw = sing.tile([128, NKC], f32)
        sb_q = sing.tile([128, N], f32r)
        sb_k = sing.tile([128, NPAD], f32r)
        sb_vt = sing.tile([128, NKC, 33], bf16)

        # x first, split for queue parallelism; weights after.
        for cc in range(2):
            for hh in range(2):
                sl = slice(hh * (NPAD // 2), (hh + 1) * (NPAD // 2))
                nc.sync.dma_start(out=sb_x[:, cc, sl], in_=xd[cc][:, sl])
        for cc in range(2):
            nc.sync.dma_start(out=sb_wkt[:, cc, :], in_=wkt[cc])
            nc.sync.dma_start(out=sb_wqt[:, cc, :], in_=wqt[cc])
            nc.sync.dma_start(out=sb_wvt[:, cc, :], in_=wvt[cc])
        nc.sync.dma_start(out=sb_pwt[:], in_=pwt[:])
        nc.sync.dma_start(out=sb_qw[:], in_=qwd[:])

        # ---- phase A helpers (emission interleaved with scores below) ----
        def q_proj(qc):
            sl = slice(qc * QCH, (qc + 1) * QCH)
            pq = ps_s.tile([128, QCH], f32, tag="s")
            for cc in range(2):
                nc.tensor.matmul(
                    pq[:, :],
                    sb_wqt[:, cc, :],
                    sb_x[:, cc, sl],
                    start=(cc == 0),
                    stop=(cc == 1),
                )
            nc.vector.tensor_copy(out=sb_q[:, sl], in_=pq[:, :])

        def k_proj(sl):
            # K projections also cover the zero-padded tail so padded-key
            # columns land as 0 (exp gives finite values, zeroed by qw=0).
            w = sl.stop - sl.start
            pk = ps_s.tile([128, QCH], f32, tag="s")
            for cc in range(2):
                nc.tensor.matmul(
                    pk[:, 0:w],
                    sb_wkt[:, cc, :],
                    sb_x[:, cc, sl],
                    start=(cc == 0),
                    stop=(cc == 1),
                )
            nc.vector.tensor_copy(out=sb_k[:, sl], in_=pk[:, 0:w])

        def v_proj():
            # V^T (pixels on partitions): copy, append ones, scale by qw so
            # the last column holds qw (weighted-rowsum denominators).
            pv = ps_s.tile([128, NKC * 32], f32, tag="s")
            for kc in range(NKC):
                for cc in range(2):
                    nc.tensor.matmul(
                        pv[:, kc * 32 : (kc + 1) * 32],
                        sb_x[:, cc, kc * 128 : (kc + 1) * 128],
                        sb_wvt[:, cc, :],
                        start=(cc == 0),
                        stop=(cc == 1),
                    )
            nc.vector.tensor_copy(
                out=sb_vt[:, :, 0:32],
                in_=pv[:, :].rearrange("p (a b) -> p a b", b=32),
            )
            nc.gpsimd.memset(sb_vt[:, :, 32:33], 1.0)
            for kc in range(NKC):
                nc.vector.tensor_scalar_mul(
                    out=sb_vt[:, kc, :],
                    in0=sb_vt[:, kc, :],
                    scalar1=sb_qw[:, kc : kc + 1],
                )

        # ---- phases B/C per query chunk ----
        et_tiles = []

        def scores_and_exp(qc):
            et = ets.tile([128, NKC, QCH], bf16, tag="et")
            qsl = slice(qc * QCH, (qc + 1) * QCH)
            for g in range(11):
                pg = ps_s.tile([128, 3, 512], f32, tag="s")
                for t in range(3):
                    kc = 3 * g + t
                    base = 32 * (kc % 4)
                    nc.tensor.matmul(
                        pg[:, t, 0:QCH],
                        sb_k[base : base + 32, kc * 128 : (kc + 1) * 128],
                        sb_q[base : base + 32, qsl],
                        tile_position=(base, 0),
                    )
                nc.scalar.activation(
                    out=et[:, 3 * g : 3 * g + 3, :],
                    in_=pg[:, :, 0:QCH],
                    func=mybir.ActivationFunctionType.Exp,
                    scale=SCALE,
                    bias=0.0,
                )
            return et

        def attnv_pair(jlo, pair):
            # col-tiled strips at partition bases 0 (qchunk jlo) and 64
            # (qchunk jlo+1) accumulating in one PSUM bank.
            po = ps_o.tile([128, 512], f32, tag="o")
            for kc in range(NKC):
                for s in range(pair):
                    base = 64 * s
                    nc.tensor.matmul(
                        po[base : base + 33, 0:QCH],
                        sb_vt[:, kc, :],
                        et_tiles[jlo + s][:, kc, :],
                        start=(kc == 0),
                        stop=(kc == NKC - 1),
                        skip_group_check=True,
                    )
            ou = ous.tile([128, QCH], f32r, tag="ou")
            for s in range(pair):
                base = 64 * s
                qc = jlo + s
                nc.vector.tensor_copy(
                    out=ou[base : base + 33, :], in_=po[base : base + 33, 0:QCH]
                )
                nc.sync.dma_start(
                    out=rd[0:1, qc * QCH : (qc + 1) * QCH],
                    in_=ou[base + 32 : base + 33, :],
                )
            # p-projection for each strip
            for s in range(pair):
                base = 64 * s
                qc = jlo + s
                for mc in range(2):
                    pu = ps_u.tile([128, 512], f32, tag="u")
                    nc.tensor.matmul(
                        pu[:, 0:QCH],
                        sb_pwt[base : base + 32, mc * 128 : (mc + 1) * 128],
                        ou[base : base + 32, :],
                    )
                    ut = us.tile([128, QCH], f32, tag="u")
                    nc.vector.tensor_copy(out=ut[:], in_=pu[:, 0:QCH])
                    nc.sync.dma_start(
                        out=ud[mc, :, qc * QCH : (qc + 1) * QCH], in_=ut[:]
                    )

        # Emission order tuned so ScalarE (exp) stays fed: K projection +
        # first Q chunk up front, remaining projections behind the first
        # score batch, attnV pairs spread between score batches.
        for sl in [slice(qc * QCH, (qc + 1) * QCH) for qc in range(NQC)] + [
            slice(N, NPAD)
        ]:
            k_proj(sl)
        q_proj(0)
        et_tiles.append(scores_and_exp(0))
        for qc in range(1, NQC):
            q_proj(qc)
        v_proj()
        et_tiles.append(scores_and_exp(1))
        et_tiles.append(scores_and_exp(2))
        attnv_pair(0, 2)
        et_tiles.append(scores_and_exp(3))
        attnv_pair(2, 2)
        et_tiles.append(scores_and_exp(4))
        et_tiles.append(scores_and_exp(5))
        attnv_pair(4, 2)
        et_tiles.append(scores_and_exp(6))
        et_tiles.append(scores_and_exp(7))
        attnv_pair(6, 2)
        et_tiles.append(scores_and_exp(8))
        attnv_pair(8, 1)

    nc.compile()
    return nc


def _host_inputs(query, q_w, k_w, v_w, p_w, q_b, k_b, log_qw):
    xf = np.ascontiguousarray(
        np.asarray(query, dtype=np.float32).reshape(C, N)
    )
    x_pad = np.zeros((2, 128, NPAD), np.float32)
    x_pad[0, :, :N] = xf[0:128]
    x_pad[1, :, :N] = xf[128:256]

    lq = np.asarray(log_qw, dtype=np.float32).reshape(N).astype(np.float64)
    lq = lq - lq.max()  # global shift cancels in U/r

    in_maps = []
    for h in range(HEADS):
        hs = slice(DK * h, DK * (h + 1))
        wq_h = np.asarray(q_w, np.float32)[hs]  # [32, 256]
        wk_h = np.asarray(k_w, np.float32)[hs]
        wv_h = np.asarray(v_w, np.float32)[hs]
        pw_h = np.asarray(p_w, np.float32)[:, hs]  # [256, 32]

        wqt = np.ascontiguousarray(np.tile(wq_h, (4, 1)).T.reshape(2, 128, 128))
        wkt = np.ascontiguousarray(np.tile(wk_h, (4, 1)).T.reshape(2, 128, 128))
        wvt = np.ascontiguousarray(wv_h.T.reshape(2, 128, 32))

        pwt = np.zeros((128, 256), np.float32)
        pwt[0:32] = pw_h.T
        pwt[64:96] = pw_h.T

        lq_h = lq
        qb_h = np.asarray(q_b, np.float64)[hs]
        if np.any(qb_h):
            Kh = (
                np.asarray(k_w, np.float64)[hs] @ xf.astype(np.float64)
                + np.asarray(k_b, np.float64)[hs][:, None]
            )
            lq_h = lq + SCALE * (qb_h @ Kh)
        qw_pad = np.zeros(NPAD, np.float64)
        qw_pad[:N] = np.exp(lq_h)
        qwd = np.ascontiguousarray(
            qw_pad.reshape(NKC, 128).T.astype(np.float32)
        )

        in_maps.append(
            {
                "x": x_pad,
                "wqt": wqt,
                "wkt": wkt,
                "wvt": wvt,
                "pwt": pwt,
                "qwd": qwd,
            }
        )
    return in_maps


def kernel(query, q_w, q_b, k_w, k_b, v_w, v_b, p_w, p_b, log_qw, _res=None):
    from concourse.bass_utils import run_bass_kernel_spmd

    if "nc" not in _cache:
        _cache["nc"] = _build_nc()
    nc = _cache["nc"]

    in_maps = _host_inputs(query, q_w, k_w, v_w, p_w, q_b, k_b, log_qw)
    res = run_bass_kernel_spmd(nc, in_maps, core_ids=list(range(8)))
    if _res is not None:
        _res.append(res)

    acc = np.zeros((C, N), np.float64)
    for h in range(HEADS):
        u = res.results[h]["u"].astype(np.float64).reshape(C, N)
        r = res.results[h]["r"].astype(np.float64).reshape(N)
        acc += u / r[None, :]

    acc += (np.asarray(p_w, np.float64) @ np.asarray(v_b, np.float64))[:, None]
    acc += np.asarray(p_b, np.float64)[:, None]
    return acc.astype(np.float32).reshape(1, C, HLAT, WLON)
